# revision 5
# baseline (speedup 1.0000x reference)
"""Trainium2 Bass kernel for nn_BlurConv2d: depthwise 11x11 box blur, reflect pad.

Approach: the (separable) 11x11 blur of each 256x256 image X is two banded
matmuls with reflection baked into 256x256 matrices built host-side:

    tmpT = X^T @ Bv        (vertical blur, transposed layout  [w, h'])
    out  = tmpT^T @ Bh     (horizontal blur, natural layout   [h', w'])

Both stages map onto nc.tensor.matmul(out, lhsT, rhs) = lhsT.T @ rhs with the
per-image data as the stationary operand (natural SBUF slices, no on-chip
transposes) and the shared Bv/Bh matrices as the moving operand.

Matmuls run in float32r (fp32 with 11-bit mantissa, 4x the fp32 streaming
rate on the PE). Bv/Bh carry raw integer tap counts {1, 2} (exact in f32r);
the 1/121 kernel scale is applied in the final PSUM->SBUF copies. The input
is pre-rounded to the f32r grid host-side, so the only error vs fp32 is
~2^-12 input/intermediate quantization (~1e-4 relative overall).

Sharding: pure data parallelism — the 16*64 = 1024 (b, c) images are split
128 per NeuronCore across 8 cores; no communication.
"""

import numpy as np

N_CORES = 8
H = 256            # image height/width
KS = 11
PAD = KS // 2
N_IMG = 16 * 64    # total (b, c) images
IMG_PER_CORE = N_IMG // N_CORES   # 128
GRP = 4            # images per DMA group
DT_NP = np.float32

_COMPILED = None   # compiled Bass module cache
LAST_RESULTS = None  # BassKernelResults of the most recent run (for profiling)


def _round_f32r(a):
    """Round fp32 array to the float32r grid (11 explicit mantissa bits)."""
    bits = np.ascontiguousarray(a, np.float32).view(np.uint32)
    return ((bits + 0x800) & np.uint32(0xFFFFF000)).view(np.float32)


def _reflect(p, n):
    if p < 0:
        return -p
    if p > n - 1:
        return 2 * (n - 1) - p
    return p


def _blur_mats(kernel2d):
    """Raw tap-count matrices (integer entries, exact in f32r) and the scale.

    Bv[h, h'] = Mv_raw[h', h], Bh[w, w'] = Mh_raw[w', w], where
    Mv_raw/Mh_raw count reflected box taps; out = (Mv_raw X Mh_raw^T) * scale.
    Only valid for a uniform (box) kernel; falls back to general separable
    taps otherwise.
    """
    k = kernel2d.astype(np.float64)
    if np.allclose(k, k.flat[0]):
        a = np.ones(KS)
        b = np.ones(KS)
        scale = float(k.flat[0])
    else:  # general rank-1 kernel
        u, s, vt = np.linalg.svd(k)
        a = u[:, 0] * np.sqrt(s[0])
        b = vt[0] * np.sqrt(s[0])
        if a.sum() < 0:
            a, b = -a, -b
        scale = 1.0
    Bv = np.zeros((H, H), np.float64)
    Bh = np.zeros((H, H), np.float64)
    for o in range(H):
        for t in range(KS):
            p = _reflect(o + t - PAD, H)
            Bv[p, o] += a[t]
            Bh[p, o] += b[t]
    return (_round_f32r(Bv.astype(np.float32)),
            _round_f32r(Bh.astype(np.float32)),
            np.float32(scale))


def _build_program():
    import concourse.bacc as bacc
    import concourse.mybir as mybir
    import concourse.tile as tile

    f32 = mybir.dt.float32
    f32r = mybir.dt.float32r
    nc = bacc.Bacc("TRN2", target_bir_lowering=False, debug=False,
                   num_devices=N_CORES)

    x_dram = nc.dram_tensor("x", [IMG_PER_CORE, H, H], f32r, kind="ExternalInput")
    bv_dram = nc.dram_tensor("bv", [H, H], f32r, kind="ExternalInput")
    bh_dram = nc.dram_tensor("bh", [H, H], f32r, kind="ExternalInput")
    sc_dram = nc.dram_tensor("sc", [128, 1], f32, kind="ExternalInput")
    y_dram = nc.dram_tensor("y", [IMG_PER_CORE, H, H], f32, kind="ExternalOutput")

    n_grp = IMG_PER_CORE // GRP

    with tile.TileContext(nc) as tc:
        with (
            tc.tile_pool(name="consts", bufs=1) as consts,
            tc.tile_pool(name="xin", bufs=3) as xin,
            tc.tile_pool(name="tmp", bufs=4) as tmp,
            tc.tile_pool(name="yout", bufs=3) as yout,
            tc.tile_pool(name="ps1", bufs=4, space="PSUM") as ps1,
            tc.tile_pool(name="ps2", bufs=4, space="PSUM") as ps2,
        ):
            bv_sb = consts.tile([128, 2, H], f32r)
            bh_sb = consts.tile([128, 2, H], f32r)
            nc.sync.dma_start(bv_sb[:], bv_dram.rearrange("(k p) n -> p k n", k=2))
            nc.sync.dma_start(bh_sb[:], bh_dram.rearrange("(k p) n -> p k n", k=2))
            # per-partition scale vector for the scaled output copies
            sc_sb = consts.tile([128, 1], f32)
            nc.sync.dma_start(sc_sb[:], sc_dram[:])

            for g in range(n_grp):
                x_sb = xin.tile([128, GRP, 2, H], f32r, tag="x")
                nc.sync.dma_start(
                    x_sb[:],
                    x_dram[g * GRP:(g + 1) * GRP].rearrange(
                        "b (k p) w -> p b k w", k=2),
                )
                y_sb = yout.tile([128, GRP, 2, H], f32, tag="y")
                for b in range(GRP):
                    # stage 1: tmpT = X^T @ Bv, psum per w-chunk r
                    t_sb = tmp.tile([128, 2, H], f32r, tag="t")
                    for r in range(2):
                        pt = ps1.tile([128, H], f32, tag="ps1")
                        for k in range(2):
                            nc.tensor.matmul(
                                pt[:],
                                x_sb[:, b, k, r * 128:(r + 1) * 128],
                                bv_sb[:, k, :],
                                start=(k == 0), stop=(k == 1),
                            )
                        # rounding copy fp32 PSUM -> f32r SBUF
                        if r == 0:
                            nc.vector.tensor_copy(t_sb[:, r, :], pt[:])
                        else:
                            nc.scalar.copy(t_sb[:, r, :], pt[:])
                    # stage 2: out = tmpT^T @ Bh, psum per h-chunk s
                    for s in range(2):
                        po = ps2.tile([128, H], f32, tag="ps2")
                        for k in range(2):
                            nc.tensor.matmul(
                                po[:],
                                t_sb[:, k, s * 128:(s + 1) * 128],
                                bh_sb[:, k, :],
                                start=(k == 0), stop=(k == 1),
                            )
                        # scaled copy applies the 1/121 kernel normalization
                        if s == 0:
                            nc.vector.tensor_scalar_mul(
                                y_sb[:, b, s, :], po[:], sc_sb[:])
                        else:
                            nc.scalar.mul(y_sb[:, b, s, :], po[:], sc_sb[:])
                nc.sync.dma_start(
                    y_dram[g * GRP:(g + 1) * GRP].rearrange(
                        "b (s p) w -> p b s w", s=2),
                    y_sb[:],
                )

    nc.compile()
    return nc


def kernel(input, kernel):
    global _COMPILED, LAST_RESULTS
    from concourse.bass_utils import run_bass_kernel_spmd

    x = _round_f32r(np.asarray(input, np.float32))
    k2d = np.asarray(kernel, np.float32)[0]
    Bv, Bh, scale = _blur_mats(k2d)

    if _COMPILED is None:
        _COMPILED = _build_program()
    nc = _COMPILED

    shards = x.reshape(N_CORES, IMG_PER_CORE, H, H)
    sc = np.full((128, 1), scale, np.float32)
    in_maps = [{"x": shards[c], "bv": Bv, "bh": Bh, "sc": sc}
               for c in range(N_CORES)]
    res = run_bass_kernel_spmd(nc, in_maps, core_ids=list(range(N_CORES)))
    LAST_RESULTS = res
    out = np.concatenate([r["y"] for r in res.results], axis=0)
    return out.reshape(np.asarray(input).shape).astype(DT_NP, copy=False)


# revision 8
# speedup vs baseline: 1.0988x; 1.0988x over previous
"""Trainium2 Bass kernel for nn_BlurConv2d: depthwise 11x11 box blur, reflect pad.

Approach: the (separable) 11x11 blur of each 256x256 image X is two banded
matmuls with reflection baked into 256x256 matrices built host-side:

    tmpT = X^T @ Bv        (vertical blur, transposed layout  [w, h'])
    out  = tmpT^T @ Bh     (horizontal blur, natural layout   [h', w'])

Both stages map onto nc.tensor.matmul(out, lhsT, rhs) = lhsT.T @ rhs with the
per-image data as the stationary operand (natural SBUF slices, no on-chip
transposes) and the shared Bv/Bh matrices as the moving operand.

Matmuls run in float32r (fp32 with 11-bit mantissa, 4x the fp32 streaming
rate on the PE). Bv/Bh carry raw integer tap counts {1, 2} (exact in f32r);
the 1/121 kernel scale is applied in the final PSUM->SBUF copies. The input
is pre-rounded to the f32r grid host-side, so the only error vs fp32 is
~2^-12 input/intermediate quantization (~1e-4 relative overall).

Sharding: pure data parallelism — the 16*64 = 1024 (b, c) images are split
128 per NeuronCore across 8 cores; no communication.
"""

import numpy as np

N_CORES = 8
H = 256            # image height/width
KS = 11
PAD = KS // 2
N_IMG = 16 * 64    # total (b, c) images
IMG_PER_CORE = N_IMG // N_CORES   # 128
GRP = 2            # images per DMA group
DT_NP = np.float32

_COMPILED = None   # compiled Bass module cache
LAST_RESULTS = None  # BassKernelResults of the most recent run (for profiling)


def _round_f32r(a):
    """Round fp32 array to the float32r grid (11 explicit mantissa bits)."""
    bits = np.ascontiguousarray(a, np.float32).view(np.uint32)
    return ((bits + 0x800) & np.uint32(0xFFFFF000)).view(np.float32)


def _reflect(p, n):
    if p < 0:
        return -p
    if p > n - 1:
        return 2 * (n - 1) - p
    return p


def _blur_mats(kernel2d):
    """Raw tap-count matrices (integer entries, exact in f32r) and the scale.

    Bv[h, h'] = Mv_raw[h', h], Bh[w, w'] = Mh_raw[w', w], where
    Mv_raw/Mh_raw count reflected box taps; out = (Mv_raw X Mh_raw^T) * scale.
    Only valid for a uniform (box) kernel; falls back to general separable
    taps otherwise.
    """
    k = kernel2d.astype(np.float64)
    if np.allclose(k, k.flat[0]):
        a = np.ones(KS)
        b = np.ones(KS)
        scale = float(k.flat[0])
    else:  # general rank-1 kernel
        u, s, vt = np.linalg.svd(k)
        a = u[:, 0] * np.sqrt(s[0])
        b = vt[0] * np.sqrt(s[0])
        if a.sum() < 0:
            a, b = -a, -b
        scale = 1.0
    Bv = np.zeros((H, H), np.float64)
    Bh = np.zeros((H, H), np.float64)
    for o in range(H):
        for t in range(KS):
            p = _reflect(o + t - PAD, H)
            Bv[p, o] += a[t]
            Bh[p, o] += b[t]
    return (_round_f32r(Bv.astype(np.float32)),
            _round_f32r(Bh.astype(np.float32)),
            np.float32(scale))


def _build_program(loops=None):
    """Build the Bass program. ``loops=K`` wraps the whole body in a
    runtime For_i loop that re-runs the full pass K times (used only by the
    differential wall-clock timing harness; the graded path uses None)."""
    from contextlib import nullcontext

    import concourse.bacc as bacc
    import concourse.mybir as mybir
    import concourse.tile as tile

    f32 = mybir.dt.float32
    f32r = mybir.dt.float32r
    nc = bacc.Bacc("TRN2", target_bir_lowering=False, debug=False,
                   num_devices=N_CORES)

    x_dram = nc.dram_tensor("x", [IMG_PER_CORE, H, H], f32r, kind="ExternalInput")
    bv_dram = nc.dram_tensor("bv", [H, H], f32r, kind="ExternalInput")
    bh_dram = nc.dram_tensor("bh", [H, H], f32r, kind="ExternalInput")
    sc_dram = nc.dram_tensor("sc", [128, 1], f32, kind="ExternalInput")
    y_dram = nc.dram_tensor("y", [IMG_PER_CORE, H, H], f32, kind="ExternalOutput")

    n_grp = IMG_PER_CORE // GRP

    with tile.TileContext(nc) as tc:
        with (
            tc.tile_pool(name="consts", bufs=1) as consts,
            tc.tile_pool(name="xin", bufs=6) as xin,
            tc.tile_pool(name="tmp", bufs=10) as tmp,
            tc.tile_pool(name="yout", bufs=6) as yout,
            tc.tile_pool(name="ps1", bufs=2, space="PSUM") as ps1,
            tc.tile_pool(name="ps2", bufs=2, space="PSUM") as ps2,
        ):
            bv_sb = consts.tile([128, 2, H], f32r)
            bh_sb = consts.tile([128, 2, H], f32r)
            nc.sync.dma_start(bv_sb[:], bv_dram.rearrange("(k p) n -> p k n", k=2))
            nc.sync.dma_start(bh_sb[:], bh_dram.rearrange("(k p) n -> p k n", k=2))
            # per-partition scale vector for the scaled output copies
            sc_sb = consts.tile([128, 1], f32)
            nc.sync.dma_start(sc_sb[:], sc_dram[:])

            loop_ctx = tc.For_i(0, loops, 1) if loops else nullcontext()
            with loop_ctx:
                _emit_body(nc, tc, n_grp, x_dram, y_dram,
                           bv_sb, bh_sb, sc_sb, xin, tmp, yout, ps1, ps2)

    nc.compile()
    return nc


def _emit_body(nc, tc, n_grp, x_dram, y_dram,
               bv_sb, bh_sb, sc_sb, xin, tmp, yout, ps1, ps2):
    import concourse.mybir as mybir

    f32 = mybir.dt.float32
    f32r = mybir.dt.float32r
    for g in range(n_grp):
        x_sb = xin.tile([128, GRP, 2, H], f32r, tag="x")
        nc.sync.dma_start(
            x_sb[:],
            x_dram[g * GRP:(g + 1) * GRP].rearrange("b (k p) w -> p b k w", k=2),
        )
        y_sb = yout.tile([128, GRP, 2, H], f32, tag="y")
        for b in range(GRP):
            # stage 1: tmpT = X^T @ Bv, psum per w-chunk r
            t_sb = tmp.tile([128, 2, H], f32r, tag="t")
            for r in range(2):
                pt = ps1.tile([128, H], f32, tag="ps1")
                for k in range(2):
                    nc.tensor.matmul(
                        pt[:],
                        x_sb[:, b, k, r * 128:(r + 1) * 128],
                        bv_sb[:, k, :],
                        start=(k == 0), stop=(k == 1),
                    )
                # rounding copy fp32 PSUM -> f32r SBUF
                if r == 0:
                    nc.vector.tensor_copy(t_sb[:, r, :], pt[:])
                else:
                    nc.scalar.copy(t_sb[:, r, :], pt[:])
            # stage 2: out = tmpT^T @ Bh, psum per h-chunk s
            for s in range(2):
                po = ps2.tile([128, H], f32, tag="ps2")
                for k in range(2):
                    nc.tensor.matmul(
                        po[:],
                        t_sb[:, k, s * 128:(s + 1) * 128],
                        bh_sb[:, k, :],
                        start=(k == 0), stop=(k == 1),
                    )
                # scaled copy applies the 1/121 kernel normalization
                if s == 0:
                    nc.vector.tensor_scalar_mul(y_sb[:, b, s, :], po[:], sc_sb[:])
                else:
                    nc.scalar.mul(y_sb[:, b, s, :], po[:], sc_sb[:])
        nc.sync.dma_start(
            y_dram[g * GRP:(g + 1) * GRP].rearrange("b (s p) w -> p b s w", s=2),
            y_sb[:],
        )


def kernel(input, kernel):
    global _COMPILED, LAST_RESULTS
    from concourse.bass_utils import run_bass_kernel_spmd

    x = _round_f32r(np.asarray(input, np.float32))
    k2d = np.asarray(kernel, np.float32)[0]
    Bv, Bh, scale = _blur_mats(k2d)

    if _COMPILED is None:
        _COMPILED = _build_program()
    nc = _COMPILED

    shards = x.reshape(N_CORES, IMG_PER_CORE, H, H)
    sc = np.full((128, 1), scale, np.float32)
    in_maps = [{"x": shards[c], "bv": Bv, "bh": Bh, "sc": sc}
               for c in range(N_CORES)]
    res = run_bass_kernel_spmd(nc, in_maps, core_ids=list(range(N_CORES)))
    LAST_RESULTS = res
    out = np.concatenate([r["y"] for r in res.results], axis=0)
    return out.reshape(np.asarray(input).shape).astype(DT_NP, copy=False)


# revision 9
# speedup vs baseline: 1.0998x; 1.0009x over previous
"""Trainium2 Bass kernel for nn_BlurConv2d: depthwise 11x11 box blur, reflect pad.

Approach: the (separable) 11x11 blur of each 256x256 image X is two banded
matmuls with reflection baked into 256x256 matrices built host-side:

    tmpT = X^T @ Bv        (vertical blur, transposed layout  [w, h'])
    out  = tmpT^T @ Bh     (horizontal blur, natural layout   [h', w'])

Both stages map onto nc.tensor.matmul(out, lhsT, rhs) = lhsT.T @ rhs with the
per-image data as the stationary operand (natural SBUF slices, no on-chip
transposes) and the shared Bv/Bh matrices as the moving operand.

Matmuls run in float32r (fp32 with 11-bit mantissa, 4x the fp32 streaming
rate on the PE). Bv/Bh carry raw integer tap counts {1, 2} (exact in f32r);
the 1/121 kernel scale is applied in the final PSUM->SBUF copies. The input
is pre-rounded to the f32r grid host-side, so the only error vs fp32 is
~2^-12 input/intermediate quantization (~1e-4 relative overall).

Sharding: pure data parallelism — the 16*64 = 1024 (b, c) images are split
128 per NeuronCore across 8 cores; no communication.
"""

import numpy as np

N_CORES = 8
H = 256            # image height/width
KS = 11
PAD = KS // 2
N_IMG = 16 * 64    # total (b, c) images
IMG_PER_CORE = N_IMG // N_CORES   # 128
GRP = 2            # images per DMA group
DT_NP = np.float32

_COMPILED = None   # compiled Bass module cache
LAST_RESULTS = None  # BassKernelResults of the most recent run (for profiling)


def _round_f32r(a):
    """Round fp32 array to the float32r grid (11 explicit mantissa bits)."""
    bits = np.ascontiguousarray(a, np.float32).view(np.uint32)
    return ((bits + 0x800) & np.uint32(0xFFFFF000)).view(np.float32)


def _reflect(p, n):
    if p < 0:
        return -p
    if p > n - 1:
        return 2 * (n - 1) - p
    return p


def _blur_mats(kernel2d):
    """Raw tap-count matrices (integer entries, exact in f32r) and the scale.

    Bv[h, h'] = Mv_raw[h', h], Bh[w, w'] = Mh_raw[w', w], where
    Mv_raw/Mh_raw count reflected box taps; out = (Mv_raw X Mh_raw^T) * scale.
    Only valid for a uniform (box) kernel; falls back to general separable
    taps otherwise.
    """
    k = kernel2d.astype(np.float64)
    if np.allclose(k, k.flat[0]):
        a = np.ones(KS)
        b = np.ones(KS)
        scale = float(k.flat[0])
    else:  # general rank-1 kernel
        u, s, vt = np.linalg.svd(k)
        a = u[:, 0] * np.sqrt(s[0])
        b = vt[0] * np.sqrt(s[0])
        if a.sum() < 0:
            a, b = -a, -b
        scale = 1.0
    Bv = np.zeros((H, H), np.float64)
    Bh = np.zeros((H, H), np.float64)
    for o in range(H):
        for t in range(KS):
            p = _reflect(o + t - PAD, H)
            Bv[p, o] += a[t]
            Bh[p, o] += b[t]
    return (_round_f32r(Bv.astype(np.float32)),
            _round_f32r(Bh.astype(np.float32)),
            np.float32(scale))


def _build_program(loops=None):
    """Build the Bass program. ``loops=K`` wraps the whole body in a
    runtime For_i loop that re-runs the full pass K times (used only by the
    differential wall-clock timing harness; the graded path uses None)."""
    from contextlib import nullcontext

    import concourse.bacc as bacc
    import concourse.mybir as mybir
    import concourse.tile as tile

    f32 = mybir.dt.float32
    f32r = mybir.dt.float32r
    nc = bacc.Bacc("TRN2", target_bir_lowering=False, debug=False,
                   num_devices=N_CORES)

    x_dram = nc.dram_tensor("x", [IMG_PER_CORE, H, H], f32r, kind="ExternalInput")
    bv_dram = nc.dram_tensor("bv", [H, H], f32r, kind="ExternalInput")
    bh_dram = nc.dram_tensor("bh", [H, H], f32r, kind="ExternalInput")
    sc_dram = nc.dram_tensor("sc", [128, 1], f32, kind="ExternalInput")
    y_dram = nc.dram_tensor("y", [IMG_PER_CORE, H, H], f32, kind="ExternalOutput")

    n_grp = IMG_PER_CORE // GRP

    with tile.TileContext(nc) as tc:
        with (
            tc.tile_pool(name="consts", bufs=1) as consts,
            tc.tile_pool(name="xin", bufs=8) as xin,
            tc.tile_pool(name="tmp", bufs=12) as tmp,
            tc.tile_pool(name="yout", bufs=8) as yout,
            tc.tile_pool(name="ps1", bufs=2, space="PSUM") as ps1,
            tc.tile_pool(name="ps2", bufs=2, space="PSUM") as ps2,
        ):
            bv_sb = consts.tile([128, 2, H], f32r)
            bh_sb = consts.tile([128, 2, H], f32r)
            nc.sync.dma_start(bv_sb[:], bv_dram.rearrange("(k p) n -> p k n", k=2))
            nc.sync.dma_start(bh_sb[:], bh_dram.rearrange("(k p) n -> p k n", k=2))
            # per-partition scale vector for the scaled output copies
            sc_sb = consts.tile([128, 1], f32)
            nc.sync.dma_start(sc_sb[:], sc_dram[:])

            loop_ctx = tc.For_i(0, loops, 1) if loops else nullcontext()
            with loop_ctx:
                _emit_body(nc, tc, n_grp, x_dram, y_dram,
                           bv_sb, bh_sb, sc_sb, xin, tmp, yout, ps1, ps2)

    nc.compile()
    return nc


def _emit_body(nc, tc, n_grp, x_dram, y_dram,
               bv_sb, bh_sb, sc_sb, xin, tmp, yout, ps1, ps2):
    import concourse.mybir as mybir

    f32 = mybir.dt.float32
    f32r = mybir.dt.float32r
    for g in range(n_grp):
        x_sb = xin.tile([128, GRP, 2, H], f32r, tag="x")
        nc.sync.dma_start(
            x_sb[:],
            x_dram[g * GRP:(g + 1) * GRP].rearrange("b (k p) w -> p b k w", k=2),
        )
        y_sb = yout.tile([128, GRP, 2, H], f32, tag="y")
        for b in range(GRP):
            # stage 1: tmpT = X^T @ Bv, psum per w-chunk r
            t_sb = tmp.tile([128, 2, H], f32r, tag="t")
            for r in range(2):
                pt = ps1.tile([128, H], f32, tag="ps1")
                for k in range(2):
                    nc.tensor.matmul(
                        pt[:],
                        x_sb[:, b, k, r * 128:(r + 1) * 128],
                        bv_sb[:, k, :],
                        start=(k == 0), stop=(k == 1),
                    )
                # rounding copy fp32 PSUM -> f32r SBUF
                if r == 0:
                    nc.vector.tensor_copy(t_sb[:, r, :], pt[:])
                else:
                    nc.scalar.copy(t_sb[:, r, :], pt[:])
            # stage 2: out = tmpT^T @ Bh, psum per h-chunk s
            for s in range(2):
                po = ps2.tile([128, H], f32, tag="ps2")
                for k in range(2):
                    nc.tensor.matmul(
                        po[:],
                        t_sb[:, k, s * 128:(s + 1) * 128],
                        bh_sb[:, k, :],
                        start=(k == 0), stop=(k == 1),
                    )
                # scaled copy applies the 1/121 kernel normalization
                if s == 0:
                    nc.vector.tensor_scalar_mul(y_sb[:, b, s, :], po[:], sc_sb[:])
                else:
                    nc.scalar.mul(y_sb[:, b, s, :], po[:], sc_sb[:])
        nc.sync.dma_start(
            y_dram[g * GRP:(g + 1) * GRP].rearrange("b (s p) w -> p b s w", s=2),
            y_sb[:],
        )


def kernel(input, kernel):
    global _COMPILED, LAST_RESULTS
    from concourse.bass_utils import run_bass_kernel_spmd

    x = _round_f32r(np.asarray(input, np.float32))
    k2d = np.asarray(kernel, np.float32)[0]
    Bv, Bh, scale = _blur_mats(k2d)

    if _COMPILED is None:
        _COMPILED = _build_program()
    nc = _COMPILED

    shards = x.reshape(N_CORES, IMG_PER_CORE, H, H)
    sc = np.full((128, 1), scale, np.float32)
    in_maps = [{"x": shards[c], "bv": Bv, "bh": Bh, "sc": sc}
               for c in range(N_CORES)]
    res = run_bass_kernel_spmd(nc, in_maps, core_ids=list(range(N_CORES)))
    LAST_RESULTS = res
    out = np.concatenate([r["y"] for r in res.results], axis=0)
    return out.reshape(np.asarray(input).shape).astype(DT_NP, copy=False)
